# revision 1
# baseline (speedup 1.0000x reference)
"""3-layer LSTM (B=256, S=1024, H=128, V=288, E=100) on 8 Trainium2 cores.

Strategy
--------
Data-parallel over batch: 8 cores x 32 batch each. Per core, the three LSTM
layers run in a chunk-skewed software pipeline (layer l lags layer l-1 by one
8-step chunk) so the per-step elementwise work of all three layers merges into
single wide ACT/DVE instructions.

Per-core layout ("gateT"): hidden dim on the 128 SBUF/PSUM partitions, batch
(32) on the free dim. The recurrent matmul for gate G is
    psum[h, b] += WhhT_G.T @ h'      (stationary = WhhT_G [128,128] fp16)

All activations use the sigmoid LUT only (tanh(x) = 2*sigmoid(2x)-1, folded
into weight scaling), so one table set is loaded once. Rescalings:
    h' = h/2   (consumers' weights doubled:   Whh, Wih_1/2, fcW)
    c' = c/2   (sigma(2c) computed as sigmoid with scale=4 immediate)
    g-gate weights/biases doubled (sigma(2g))
Gate math per tick (merged over active layers, fp16):
    sfiog = sigmoid(gates_psum)                     1 ACT op  [128, 4*3*32]
    sc    = sigmoid(4*c'_{t-1})                     1 ACT op  [128, 3*32]
    h'    = (sc - 0.5) * so                         1 DVE STT -> hist
    m2    = (sg - 0.5) * si                         1 DVE STT (= si*tanh(g)/2)
    m1    = sf * c'_{t-1}                           1 DVE TT
    c'    = m1 + m2                                 1 DVE TT
Layer-0 input projection: precomputed table T0 = emb@Wih0.T + b0 [288, 512]
(fp16, in DRAM); rows gathered per chunk by indirect DMA, injected into PSUM
via identity matmuls. Layers 1/2 input projections: bulk matmuls from the h'
history ring once per chunk. Biases: rank-4 matmul (biasmat x selector).
FC head: per 4 steps, one matmul [128(h),128(s,b)]x[128,288] + rank-1 bias,
DVE copy to fp16, SWDGE cast-DMA to the fp32 output.

PSUM: one [128, 3072] fp32 tile = 6 banks (3 layers x 2 step-parity banks,
each bank = [G(4)][q(4)][b(32)]), consecutive steps alternate banks so PE
writes and ACT reads never collide on a bank. +2 banks for the FC head.
"""

import os
import numpy as np
from contextlib import ExitStack

import concourse.bass as bass
import concourse.tile as tile
from concourse import bacc, mybir
from concourse.bass import IndirectOffsetOnAxis
from concourse.bass_utils import run_bass_kernel_spmd
from concourse.tile_rust import add_dep_helper


def _raw(inst):
    return getattr(inst, "ins", inst)


def _dep(mm_inst, *producers):
    """Tile does not track the matmul stationary (lhsT) operand as a read;
    add the producer->matmul edges explicitly."""
    for p in producers:
        if p is not None:
            add_dep_helper(_raw(mm_inst), _raw(p), True, "lhsT producer")

F16 = mybir.dt.float16
F32 = mybir.dt.float32
I32 = mybir.dt.int32

B, S, V, E, H = 256, 1024, 288, 100, 128
NCORES = 8
BS = B // NCORES          # 32 batch per core
CH = 8                    # steps per chunk (pipeline skew unit)
# device gate order: 0:i 1:f 2:o 3:g   (reference order is i,f,g,o)
REF_BLOCK = {0: 0, 1: 1, 2: 3, 3: 2}
SIG = mybir.ActivationFunctionType.Sigmoid
MUL = mybir.AluOpType.mult
ADD = mybir.AluOpType.add
SUB = mybir.AluOpType.subtract


def _prep_params(inputs, nsteps):
    """Host-side parameter preprocessing (scaling, transposes, fp16)."""
    f32 = np.float32
    emb = np.asarray(inputs["emb"], f32)
    p = {}

    # Layer 0 table: T0[v, G*128+m] = (emb @ Wih0.T + bih0 + bhh0), device
    # gate order, g-gate doubled. No h-scaling (input is the embedding).
    Wih0 = np.asarray(inputs["Wih0"], f32)
    b0 = np.asarray(inputs["bih0"], f32) + np.asarray(inputs["bhh0"], f32)
    T0full = emb @ Wih0.T + b0                      # [V, 4H] ref order
    T0 = np.empty((V, 4 * H), f32)
    for G in range(4):
        blk = REF_BLOCK[G]
        scale = 2.0 if G == 3 else 1.0
        T0[:, G * H:(G + 1) * H] = T0full[:, blk * H:(blk + 1) * H] * scale
    p["T0tab"] = T0.astype(np.float16)

    # Recurrent weights, all layers: WhhT[:, l*512 + G*128 + m]
    # = Whh_l[refblk(G)*128 + m, :].T * 2 * (2 if G==g)
    WhhT = np.empty((H, 3 * 4 * H), f32)
    for l in range(3):
        Whh = np.asarray(inputs[f"Whh{l}"], f32)
        for G in range(4):
            blk = REF_BLOCK[G]
            scale = 2.0 * (2.0 if G == 3 else 1.0)
            WhhT[:, l * 512 + G * H:l * 512 + (G + 1) * H] = \
                Whh[blk * H:(blk + 1) * H, :].T * scale
    p["WhhT"] = WhhT.astype(np.float16)

    # Input-projection weights for layers 1,2 (consume h -> x2 scaling).
    WihT = np.empty((H, 2 * 4 * H), f32)
    for l in (1, 2):
        Wih = np.asarray(inputs[f"Wih{l}"], f32)
        for G in range(4):
            blk = REF_BLOCK[G]
            scale = 2.0 * (2.0 if G == 3 else 1.0)
            WihT[:, (l - 1) * 512 + G * H:(l - 1) * 512 + (G + 1) * H] = \
                Wih[blk * H:(blk + 1) * H, :].T * scale
    p["WihT"] = WihT.astype(np.float16)

    # biases for layers 1,2: biasmat[G, (l-1)*128 + m]
    bias = np.empty((4, 2 * H), f32)
    for l in (1, 2):
        bl = np.asarray(inputs[f"bih{l}"], f32) + np.asarray(inputs[f"bhh{l}"], f32)
        for G in range(4):
            blk = REF_BLOCK[G]
            scale = 2.0 if G == 3 else 1.0
            bias[G, (l - 1) * H:l * H] = bl[blk * H:(blk + 1) * H] * scale
    p["BIASM"] = bias.astype(np.float16)

    # selector [4, 512]: sel[k, n] = (n // 128 == k)
    sel = (np.arange(512)[None, :] // H == np.arange(4)[:, None])
    p["SELM"] = sel.astype(np.float16)
    p["I128"] = np.eye(H, dtype=np.float16)
    p["FCWT"] = (np.asarray(inputs["fcW"], f32).T * 2.0).astype(np.float16)  # [H, V]
    p["ONES"] = np.ones((1, H), np.float16)
    p["FCB"] = np.asarray(inputs["fcb"], f32)[None, :].astype(np.float16)   # [1, V]
    return p


def _prep_text(text_slice, nsteps):
    """[BS, nsteps] int32 -> [128, nsteps//4] with partition=(q*32+b), col=(2c+p),
    value text[b, 8c+2q+p]."""
    nph = nsteps // CH
    t = np.asarray(text_slice, np.int32).reshape(BS, nph, 4, 2)  # [b, c, q, p]
    t = t.transpose(2, 0, 1, 3).reshape(4 * BS, nph * 2)         # [(q b), (c p)]
    return np.ascontiguousarray(t)


def build_module(nsteps=S, debug=False, hostgather=False):
    """Emit the Bass/Tile module for one core. Returns (nc, input_names)."""
    NPH = nsteps // CH
    nc = bacc.Bacc("TRN2", target_bir_lowering=False)
    d_slabs = None
    if hostgather:
        d_slabs = nc.dram_tensor("slabs", [2 * NPH, 4 * BS, 4 * H], F16,
                                 kind="ExternalInput")
    dbg = {}
    if debug:
        dbg["slab00"] = nc.dram_tensor("dbg_slab00", [4 * BS, 4 * H], F16, kind="ExternalOutput")
        dbg["sf00"] = nc.dram_tensor("dbg_sf00", [H, 4 * 3 * BS], F16, kind="ExternalOutput")
        dbg["hist"] = nc.dram_tensor("dbg_hist", [H, 2 * 3 * CH * BS], F16, kind="ExternalOutput")
        dbg["cp00"] = nc.dram_tensor("dbg_cp00", [H, 2 * 3 * BS], F16, kind="ExternalOutput")

    d_text = nc.dram_tensor("text2", [4 * BS, 2 * NPH], I32, kind="ExternalInput")
    d_t0 = nc.dram_tensor("T0tab", [V, 4 * H], F16, kind="ExternalInput")
    d_whh = nc.dram_tensor("WhhT", [H, 12 * H], F16, kind="ExternalInput")
    d_wih = nc.dram_tensor("WihT", [H, 8 * H], F16, kind="ExternalInput")
    d_bias = nc.dram_tensor("BIASM", [4, 2 * H], F16, kind="ExternalInput")
    d_sel = nc.dram_tensor("SELM", [4, 4 * H], F16, kind="ExternalInput")
    d_i128 = nc.dram_tensor("I128", [H, H], F16, kind="ExternalInput")
    d_fcw = nc.dram_tensor("FCWT", [H, V], F16, kind="ExternalInput")
    d_ones = nc.dram_tensor("ONES", [1, H], F16, kind="ExternalInput")
    d_fcb = nc.dram_tensor("FCB", [1, V], F16, kind="ExternalInput")
    d_out = nc.dram_tensor("out", [nsteps, BS, V], F32, kind="ExternalOutput")
    out2d = d_out[:].rearrange("s b v -> (s b) v")

    with tile.TileContext(nc) as tc, ExitStack() as ctx:
        cpool = ctx.enter_context(tc.tile_pool(name="const", bufs=1))
        spool = ctx.enter_context(tc.tile_pool(name="state", bufs=1))
        gpool = ctx.enter_context(tc.tile_pool(name="slabs", bufs=4))
        apool = ctx.enter_context(tc.tile_pool(name="acts", bufs=3))
        opool = ctx.enter_context(tc.tile_pool(name="fcout", bufs=3))
        pgate = ctx.enter_context(tc.tile_pool(name="pgate", bufs=1, space="PSUM"))
        pfc = ctx.enter_context(tc.tile_pool(name="pfc", bufs=2, space="PSUM"))

        # ---- constants in SBUF ----
        TXT = cpool.tile([4 * BS, 2 * NPH], I32)
        WHH = cpool.tile([H, 12 * H], F16)
        WIH = cpool.tile([H, 8 * H], F16)
        BIA = cpool.tile([4, 2 * H], F16)
        SEL = cpool.tile([4, 4 * H], F16)
        IDN = cpool.tile([H, H], F16)
        FCW = cpool.tile([H, V], F16)
        ONE = cpool.tile([1, H], F16)
        FCB = cpool.tile([1, V], F16)
        ld = {}
        for nm, t_, d_ in (("TXT", TXT, d_text), ("WHH", WHH, d_whh),
                           ("WIH", WIH, d_wih), ("BIA", BIA, d_bias),
                           ("SEL", SEL, d_sel), ("IDN", IDN, d_i128),
                           ("FCW", FCW, d_fcw), ("ONE", ONE, d_ones),
                           ("FCB", FCB, d_fcb)):
            ld[nm] = nc.sync.dma_start(t_[:], d_[:])

        # ---- state ----
        # h' history ring: [128, buf(2) * (l(3) * s(8) * b(32))]
        HIST = spool.tile([H, 2 * 3 * CH * BS], F16)
        # c' ping-pong: [128, par(2) * (l(3) * b(32))]
        CP = spool.tile([H, 2 * 3 * BS], F16)
        hist_memset = nc.gpsimd.memset(HIST[:], 0.0)
        nc.gpsimd.memset(CP[:], 0.0)
        # producer of each hist slot (buf, j) for explicit lhsT deps
        hw = {(u, j): hist_memset for u in (0, 1) for j in range(CH)}
        hist = HIST[:].rearrange("x (u l s b) -> x u l s b", u=2, l=3, s=CH, b=BS)

        # gates psum: [128, l(3)*p(2)*G(4)*q(4)*b(32)] = 6 banks
        GATES = pgate.tile([H, 3 * 2 * 4 * 4 * BS], F32, space="PSUM")
        gat = GATES[:].rearrange("x (l p g q b) -> x l p g q b",
                                 l=3, p=2, g=4, q=4, b=BS)

        def pe_fence(src_ap, out_ap):
            """Order all later PE instructions after src's producer.

            Tile tracks matmul *moving*-operand reads but loses the
            stationary-operand (lhsT) wait during semaphore emission, so a
            matmul consuming a freshly written tile as lhsT can run early.
            This dummy 1xN matmul reads src as the moving operand (tracked);
            the PE sequencer is in-order, so everything behind it is safe.
            """
            nc.tensor.matmul(out=out_ap, lhsT=ONE[:, 0:1], rhs=src_ap,
                             start=True, stop=True, skip_group_check=True)

        # fences for DMA-loaded tiles that are consumed as lhsT
        pe_fence(ONE[0:1, 0:1], GATES[0:1, 0:1])
        pe_fence(WHH[0:1, 0:1], GATES[0:1, 0:1])
        pe_fence(WIH[0:1, 0:1], GATES[0:1, 0:1])
        pe_fence(BIA[0:1, 0:1], GATES[0:1, 0:1])

        # per-(phase%2, parity) gather slabs
        slab = {}

        sf_prev = apool.tile([H, 4 * 3 * BS], F16, tag="sfiog")
        nc.gpsimd.memset(sf_prev[:], 0.0)

        def gather(phase):
            for par in (0, 1):
                sl = gpool.tile([4 * BS, 4 * H], F16, tag="slab")
                if hostgather:
                    gi = nc.sync.dma_start(sl[:], d_slabs[2 * phase + par])
                else:
                    gi = nc.gpsimd.indirect_dma_start(
                        out=sl[:], out_offset=None, in_=d_t0[:],
                        in_offset=IndirectOffsetOnAxis(
                            ap=TXT[:, 2 * phase + par:2 * phase + par + 1], axis=0),
                    )
                slab[(phase % 2, par)] = (sl, gi)

        gather(0)

        for C in range(NPH + 3):
            buf, pbuf = C & 1, 1 - (C & 1)
            g_lo = max(0, C - NPH + 1)
            g_hi = min(2, C)          # inclusive; gates-active layers
            g_n = g_hi - g_lo + 1

            for j in range(CH):
                par = j & 1
                q = j >> 1

                # ---- sigma(c'_{t-1}) and h'_{t-1} (write hist) ----
                if j == 0:
                    h_lo, h_hi = max(0, C - NPH), min(2, C - 1)
                    jh, bufh = CH - 1, pbuf
                else:
                    h_lo, h_hi = g_lo, g_hi
                    jh, bufh = j - 1, buf
                h_n = h_hi - h_lo + 1
                if h_n > 0:
                    sc = apool.tile([H, 3 * BS], F16, tag="sc")
                    nc.scalar.activation(
                        sc[:, :h_n * BS],
                        CP[:, (1 - par) * 96 + h_lo * BS:(1 - par) * 96 + (h_hi + 1) * BS],
                        SIG, bias=0.0, scale=4.0)
                    hw[(bufh, jh)] = nc.vector.scalar_tensor_tensor(
                        out=hist[:, bufh, h_lo:h_hi + 1, jh, :],
                        in0=sc[:].rearrange("x (l b) -> x l b", l=3)[:, :h_n, :],
                        scalar=0.5,
                        in1=sf_prev[:].rearrange("x (g l b) -> x g l b", g=4, l=3)
                            [:, 2, h_lo:h_hi + 1, :],
                        op0=SUB, op1=MUL)

                # ---- chunk-granular work, once per phase ----
                # Placed at tick 0 between the h' write (which completes the
                # previous chunk's history) and this tick's Whh matmuls
                # (which accumulate onto the freshly initialized banks).
                if j == 0:
                    X = C - 3
                    fc_on = 0 <= X <= NPH - 1
                    # FC psum tiles double as fence targets (cols 288-511 dead)
                    ps0 = pfc.tile([H, 512], F32, space="PSUM", tag="fcp")
                    ps1 = pfc.tile([H, 512], F32, space="PSUM", tag="fcp")
                    if C <= NPH - 1:
                        for p_ in (0, 1):
                            pe_fence(slab[(C % 2, p_)][0][0:1, 0:1],
                                     ps0[0:1, 448:449])
                    if fc_on:
                        bufx = (X + 2) & 1
                        for half in (0, 1):
                            pe_fence(
                                HIST[0:1, bufx * 768 + 512 + half * 128:
                                     bufx * 768 + 640 + half * 128],
                                ps0[0:1, 288:416])
                    # bias + bulk Wih for layers 1,2
                    for l in range(max(1, g_lo), g_hi + 1):
                        for p_ in (0, 1):
                            _dep(nc.tensor.matmul(
                                out=gat[:, l, p_, :, :, :],
                                lhsT=BIA[:, (l - 1) * H:l * H],
                                rhs=SEL[:],
                                start=True, stop=False, skip_group_check=True),
                                ld["BIA"])
                        for G in range(4):
                            for p_ in (0, 1):
                                _dep(nc.tensor.matmul(
                                    out=gat[:, l, p_, G, :, :],
                                    lhsT=WIH[:, (l - 1) * 512 + G * H:(l - 1) * 512 + (G + 1) * H],
                                    rhs=hist[:, pbuf, l - 1, p_::2, :],
                                    start=False, stop=False, skip_group_check=True),
                                    ld["WIH"])
                    # layer-0 xg injection via identity matmuls
                    if C <= NPH - 1:
                        for p_ in (0, 1):
                            sl, gi = slab[(C % 2, p_)]
                            for G in range(4):
                                _dep(nc.tensor.matmul(
                                    out=gat[:, 0, p_, G, :, :],
                                    lhsT=sl[:, G * H:(G + 1) * H],
                                    rhs=IDN[:],
                                    start=(G == 0), stop=False, skip_group_check=True),
                                    gi)
                    if C + 1 <= NPH - 1:
                        gather(C + 1)
                    # FC head for layer-2 chunk C-3 (completed at tick 0)
                    if fc_on:
                        for g2, ps in ((0, ps0), (1, ps1)):
                            _dep(nc.tensor.matmul(out=ps[:, :V], lhsT=ONE[:], rhs=FCB[:],
                                                  start=True, stop=False), ld["ONE"])
                            _dep(nc.tensor.matmul(
                                out=ps[:, :V],
                                lhsT=HIST[:, bufx * 768 + 2 * 256 + g2 * 128:
                                          bufx * 768 + 2 * 256 + (g2 + 1) * 128],
                                rhs=FCW[:], start=False, stop=True),
                                *[hw[(bufx, s)] for s in range(4 * g2, 4 * g2 + 4)])
                            fo = opool.tile([H, V], F16, tag="fco")
                            nc.vector.tensor_copy(fo[:], ps[:, :V])
                            s0 = CH * X + 4 * g2
                            nc.gpsimd.dma_start(
                                out=out2d[s0 * BS:(s0 + 4) * BS, :], in_=fo[:])

                # ---- per-tick recurrent matmuls ----
                if g_n > 0:
                    for l in range(g_lo, g_hi + 1):
                        rj, rbuf = (j - 1, buf) if j > 0 else (CH - 1, pbuf)
                        for G in range(4):
                            _dep(nc.tensor.matmul(
                                out=gat[:, l, par, G, q, :],
                                lhsT=WHH[:, l * 512 + G * H:l * 512 + (G + 1) * H],
                                rhs=hist[:, rbuf, l, rj, :],
                                start=False,
                                stop=(q == 3 and G == 3),
                                skip_group_check=True),
                                ld["WHH"])

                    # ---- merged sigmoid over gates ----
                    sf = apool.tile([H, 4 * 3 * BS], F16, tag="sfiog")
                    sf4 = sf[:].rearrange("x (g l b) -> x g l b", g=4, l=3)
                    nc.scalar.activation(
                        sf4[:, :, g_lo:g_hi + 1, :],
                        gat[:, g_lo:g_hi + 1, par, :, q, :].rearrange(
                            "x l g b -> x g l b"),
                        SIG, bias=0.0, scale=1.0)

                    # ---- DVE cell update ----
                    m2 = apool.tile([H, 3 * BS], F16, tag="m2")
                    m1 = apool.tile([H, 3 * BS], F16, tag="m1")
                    a, b_ = g_lo * BS, (g_hi + 1) * BS
                    nc.vector.scalar_tensor_tensor(
                        out=m2[:, a:b_], in0=sf[:, 3 * 96 + a:3 * 96 + b_],
                        scalar=0.5, in1=sf[:, a:b_], op0=SUB, op1=MUL)
                    nc.vector.tensor_tensor(
                        out=m1[:, a:b_], in0=sf[:, 96 + a:96 + b_],
                        in1=CP[:, (1 - par) * 96 + a:(1 - par) * 96 + b_], op=MUL)
                    nc.vector.tensor_tensor(
                        out=CP[:, par * 96 + a:par * 96 + b_],
                        in0=m1[:, a:b_], in1=m2[:, a:b_], op=ADD)
                    if debug and C == 0 and j == 0:
                        nc.gpsimd.dma_start(out=dbg["slab00"][:], in_=slab[(0, 0)][0][:])
                        nc.gpsimd.dma_start(out=dbg["sf00"][:], in_=sf[:])
                        nc.gpsimd.dma_start(out=dbg["cp00"][:], in_=CP[:])
                    if debug and C == 1 and j == 0:
                        nc.gpsimd.dma_start(out=dbg["hist"][:], in_=HIST[:])
                    sf_prev = sf

    nc.compile()
    return nc


_CACHE = {}


def _get_module(nsteps):
    if nsteps not in _CACHE:
        _CACHE[nsteps] = build_module(nsteps)
    return _CACHE[nsteps]


def kernel(**inputs):
    nsteps = int(inputs.get("_nsteps", S))
    run_kw = inputs.pop("_run_kw", {}) if "_run_kw" in inputs else {}
    inputs.pop("_nsteps", None)
    text = np.asarray(inputs["text"], np.int32)

    params = _prep_params(inputs, nsteps)
    nc = _get_module(nsteps)

    in_maps = []
    for c in range(NCORES):
        m = dict(params)
        m["text2"] = _prep_text(text[c * BS:(c + 1) * BS, :nsteps], nsteps)
        in_maps.append(m)

    res = run_bass_kernel_spmd(nc, in_maps, core_ids=list(range(NCORES)), **run_kw)
    out = np.concatenate([r["out"] for r in res.results], axis=1)  # [S, B, V]
    kernel.last_results = res
    return out

